# revision 90
# baseline (speedup 1.0000x reference)
"""Trainium2 Bass kernel for nn_BasicDecoder (cross-attention + MLP decoder block).

v3. Sharding: 8 cores; core c owns batch b = c//2 and head-group g = c%2
(4 heads). Reference's raw view reshape [B,H,Q,DH]->[B,Q,H*DH] makes output
row-block j depend only on head j, so each core computes a disjoint
[4096, 512] slice of the final output with no cross-core communication.

Differences vs v2:
 - All LN stats computed via DVE chunk-trees + a single ones128 matmul per
   stat (output arrives replicated across partitions): removes ~500 M=1
   stat/replicate matmuls from the PE.
 - rsqrt via DVE-only bit-trick + 2 Newton iterations: no Sqrt/Square ACT
   tables anywhere, so the whole kernel uses only the Exp and Gelu table
   sets (2 LoadActFuncSet per rep).
 - q-LN stats merged into the attention loop (single qT pass, no pre-stage).
 - Attention scores land in 2-bank PSUM pairs; one Exp activation per pair
   (halves ACT instruction overhead). Softmax denominator via one matmul on
   a DVE-summed p8 tile.
 - Weight DMAs chunked + issued just-in-time on the scalar queue so each
   stage's first matmuls are not DMA-gated; xn/ao staging double-buffered.
"""
import numpy as np
import ml_dtypes

import concourse.bass as bass
import concourse.tile as tile
from concourse import bacc, mybir
from concourse import bass_utils

F32 = mybir.dt.float32
FP16 = mybir.dt.float16
U32 = mybir.dt.uint32
AF = mybir.ActivationFunctionType
ALU = mybir.AluOpType

B, Q, KV, D, H = 4, 8192, 1024, 1024, 8
DH = D // H            # 128
OUT_C = 512
HID = 4096
EPS = 1e-5
N_CORES = 8
HPC = H // 2           # heads per core = 4
ROWS = Q // 2          # output rows per core = 4096
SUB = 512
NSUB = Q // SUB        # 16 qtok subtiles
NSTRIP = 2             # strips of 4096 qtok
MAGIC = 0x5F3759DF     # rsqrt bit-trick constant

_CACHE = {}


def _query_perm():
    """perm[P] = original qtok index at permuted position P."""
    s = np.arange(NSTRIP)[:, None, None]
    u = np.arange(8)[None, :, None]
    rho = np.arange(SUB)[None, None, :]
    return (4096 * s + 8 * rho + u).reshape(-1)


def build(nrep=1, qbias=False, stages=None):
    nc = bacc.Bacc("TRN2", target_bir_lowering=False, debug=False,
                   enable_asserts=False)

    def din(name, shape, dt=FP16):
        return nc.dram_tensor(name, shape, dt, kind="ExternalInput").ap()

    qT = din("qT", [D, Q])
    zT = din("zT", [D, KV])
    wq = din("wq", [D, 512]); wk = din("wk", [D, 512]); wv = din("wv", [D, 512])
    nwq = din("nwq", [128, HPC], F32)
    wo = din("wo", [D, D])
    w1 = din("w1", [D, HID], FP16)
    w2 = din("w2", [HID, D], FP16)
    wf = din("wf", [D, OUT_C])
    bvb = din("bvb", [128, 512], F32)
    bo = din("bo", [128, 8], F32)
    b1 = din("b1", [128, 32], F32)
    b2 = din("b2", [128, 8], F32)
    bfp = din("bfp", [128, 4], F32)
    if qbias:
        bqh = din("bqh", [128, HPC], F32)

    outT = nc.dram_tensor("outT", [OUT_C, ROWS], FP16, kind="ExternalOutput").ap()

    # [p, c, t] views of [D, N] dram tensors (D = 8 chunks x 128 partitions)
    qTv = qT.rearrange("(c p) t -> p c t", p=128)
    zTv = zT.rearrange("(c p) t -> p c t", p=128)
    wqv = wq.rearrange("(c p) n -> c p n", p=128)
    wkv = wk.rearrange("(c p) n -> c p n", p=128)
    wvv = wv.rearrange("(c p) n -> c p n", p=128)
    wov = wo.rearrange("(c p) n -> c p n", p=128)
    w1v = w1.rearrange("(c p) n -> p c n", p=128)
    w2v = w2.rearrange("(c p) n -> c p n", p=128)
    wfv = wf.rearrange("(c p) n -> c p n", p=128)

    with tile.TileContext(nc) as tc:
        with tc.tile_pool(name="outer", bufs=1) as outer, \
             tc.tile_pool(name="dstg", bufs=1, space="DRAM") as dstg:
            # ---- constants & biases ----
            ones128 = outer.tile([128, 128], FP16)
            nc.gpsimd.memset(ones128[:], 1.0)
            ones_col = outer.tile([128, 1], FP16)
            nc.vector.tensor_copy(ones_col[:], ones128[:, 0:1])
            ones_row = outer.tile([1, 128], FP16)
            nc.vector.tensor_copy(ones_row[:], ones128[0:1, :])
            nwq_t = outer.tile([128, HPC], F32); nc.sync.dma_start(nwq_t[:], nwq)
            bvb_t = outer.tile([128, 512], F32); nc.sync.dma_start(bvb_t[:], bvb)
            bo_t = outer.tile([128, 8], F32); nc.sync.dma_start(bo_t[:], bo)
            b1_t = outer.tile([128, 32], F32); nc.sync.dma_start(b1_t[:], b1)
            b2_t = outer.tile([128, 8], F32); nc.sync.dma_start(b2_t[:], b2)
            bf_t = outer.tile([128, 4], F32); nc.sync.dma_start(bf_t[:], bfp)
            if qbias:
                bq_t = outer.tile([128, HPC], F32)
                nc.sync.dma_start(bq_t[:], bqh)
                bq16 = outer.tile([128, HPC], FP16)
                nc.vector.tensor_copy(bq16[:], bq_t[:])

            ao_stg = dstg.tile([8, 128, 8, SUB], FP16)
            xn_stg = dstg.tile([8, 128, 8, SUB], FP16)

            def rsqrt_rows(pool, ve, width, tag="rs"):
                """r = 1/sqrt(ve) on [1, width] rows, DVE-only
                (bit-trick + 2 Newton iterations). Returns fp16 [1, width]."""
                # DVE arith on u32 routes through fp32, so do the magic
                # subtraction on *values* in fp32 (exact to ~64 ulp of the
                # bit pattern -- irrelevant for a Newton initial guess).
                u = ve[:].bitcast(U32)
                ush = pool.tile([1, width], U32, tag=tag + "u0")
                nc.vector.tensor_scalar(ush[:], u, 1, None,
                                        op0=ALU.logical_shift_right)
                usf = pool.tile([1, width], F32, tag=tag + "u1")
                nc.vector.tensor_copy(usf[:], ush[:])
                y0f = pool.tile([1, width], F32, tag=tag + "u2")
                nc.vector.tensor_scalar(y0f[:], usf[:], -1.0, float(MAGIC),
                                        op0=ALU.mult, op1=ALU.add)
                y0u = pool.tile([1, width], U32, tag=tag + "u3")
                nc.vector.tensor_copy(y0u[:], y0f[:])
                y = y0u[:].bitcast(F32)
                out = None
                for it in range(2):
                    y2 = pool.tile([1, width], F32, tag=tag + "t0")
                    nc.vector.tensor_tensor(y2[:], y, y, op=ALU.mult)
                    t = pool.tile([1, width], F32, tag=tag + "t1")
                    nc.vector.tensor_tensor(t[:], ve[:], y2[:], op=ALU.mult)
                    s = pool.tile([1, width], F32, tag=tag + "t2")
                    nc.vector.tensor_scalar(s[:], t[:], -0.5, 1.5,
                                            op0=ALU.mult, op1=ALU.add)
                    if it == 0:
                        yn = pool.tile([1, width], F32, tag=tag + "y")
                        nc.vector.tensor_tensor(yn[:], y, s[:], op=ALU.mult)
                        y = yn[:]
                    else:
                        out = pool.tile([1, width], FP16, tag=tag + "o")
                        nc.vector.tensor_tensor(out[:], y, s[:], op=ALU.mult)
                return out

            def stat_square(pool, src, nch, width, tag):
                """sum(x^2) prep: one DVE square op over [128, nch, width]."""
                sq = pool.tile([128, nch, width], FP16, tag=tag + "sq")
                nc.vector.tensor_tensor(sq[:], src, src, op=ALU.mult)
                return sq

            def stat_mms(ps_stat, src, sq, nch, width):
                """Partition+chunk sums of src and sq via 2x nch M=1
                accumulating matmuls into one packed PSUM bank. The two
                chains alternate so they land in different PE col-groups
                (out partitions 0 vs 32) and overlap in the array."""
                st = ps_stat.tile([64, width], F32, tag="stat")
                for c in range(nch):
                    nc.tensor.matmul(st[0:1, :], ones_col[:], src[:, c],
                                     start=(c == 0), stop=(c == nch - 1),
                                     skip_group_check=True)
                    nc.tensor.matmul(st[32:33, :], ones_col[:], sq[:, c],
                                     start=(c == 0), stop=(c == nch - 1),
                                     skip_group_check=True)
                return st

            def stat_rowmath(pool, st, n_feat, width, tag, rows="rm"):
                """Row math + Newton rsqrt on [1, width]; returns the two
                requested rows (rows="rm": rr,m2 | "ur": mu,rr)."""
                mu = pool.tile([1, width], FP16, tag=tag + "mu")
                nc.vector.tensor_scalar_mul(mu[:], st[0:1, :], 1.0 / n_feat)
                ve0 = pool.tile([1, width], F32, tag=tag + "v0")
                nc.vector.tensor_scalar(ve0[:], st[32:33, :], 1.0 / n_feat,
                                        EPS, op0=ALU.mult, op1=ALU.add)
                mu2 = pool.tile([1, width], F32, tag=tag + "m2")
                nc.vector.tensor_tensor(mu2[:], mu[:], mu[:], op=ALU.mult)
                ve = pool.tile([1, width], F32, tag=tag + "v1")
                nc.vector.tensor_tensor(ve[:], ve0[:], mu2[:], op=ALU.subtract)
                rr = rsqrt_rows(pool, ve, width, tag=tag + "r")
                if rows == "rm":
                    m2 = pool.tile([1, width], FP16, tag=tag + "mm")
                    nc.vector.tensor_tensor(m2[:], mu[:], rr[:], op=ALU.mult)
                    return rr, m2
                return mu, rr

            def stat_repl(ps_rep, out_pool, rowA, rowB, width, tag):
                """Replicate two stat rows across partitions via matmul.
                Returns [128, 2, width] fp16 (row0=rowA, row1=rowB)."""
                rep = ps_rep.tile([128, 2, width], F32, tag="sc")
                nc.tensor.matmul(rep[:, 0], ones_row[:], rowA[:],
                                 start=True, stop=True)
                nc.tensor.matmul(rep[:, 1], ones_row[:], rowB[:],
                                 start=True, stop=True)
                out16 = out_pool.tile([128, 2, width], FP16, tag=tag + "rep")
                nc.vector.tensor_copy(out16[:], rep[:])
                return out16

            def stat_rows(pool, ps_stat, ps_rep, out_pool, src, nch,
                          width, tag, rows="rm"):
                sq = stat_square(pool, src, nch, width, tag)
                st = stat_mms(ps_stat, src, sq[:], nch, width)
                ra, rb = stat_rowmath(pool, st, 128 * nch, width, tag, rows)
                return stat_repl(ps_rep, out_pool, ra, rb, width, tag)

            for _rep in range(nrep):
              with tc.tile_pool(name="pers", bufs=1) as pers:
                O_str = {}
                for s in range(NSTRIP):
                    for h in range(HPC):
                        ostr_tile = pers.tile([128, 4096], FP16,
                                              tag=f"o{s}{h}")
                        O_str[(s, h)] = ostr_tile

                with tc.tile_pool(name="kvw", bufs=1) as kvw, \
                     tc.tile_pool(name="attpre", bufs=1) as attpre, \
                     tc.tile_pool(name="psP", bufs=1, space="PSUM") as psP, \
                     tc.tile_pool(name="psSC", bufs=2, space="PSUM") as psSC, \
                     tc.tile_pool(name="psOps", bufs=1, space="PSUM") as psOps, \
                     tc.tile_pool(name="psStat", bufs=1, space="PSUM") as psStat, \
                     tc.tile_pool(name="psDen", bufs=1, space="PSUM") as psDen:
                    # [128,512] f32 accumulators alternate between the two
                    # single-buf pools (P/den) for cheap double-buffering
                    def acc_ps(idx):
                        if idx % 2 == 0:
                            acc = psP.tile([128, 512], F32, tag="P")
                        else:
                            acc = psDen.tile([128, 512], F32, tag="den")
                        return acc
                    wq_sb = kvw.tile([128, 8, 512], FP16, tag="wq")
                    K_sb = kvw.tile([128, HPC, KV], FP16, tag="K")
                    V_sb = kvw.tile([128, 8, 512], FP16, tag="V")
                    wo_sb = kvw.tile([128, 8, D], FP16, tag="wo")
                    if qbias:
                        c1_sb = kvw.tile([128, HPC, 8], F32, tag="c1")

                    # ================= KV stage =================
                    with nc.named_scope("kv"), \
                         tc.tile_pool(name="kvp", bufs=1) as kvp:
                        wk_sb = kvp.tile([128, 8, 512], FP16, tag="wk")
                        wv_sb = kvp.tile([128, 8, 512], FP16, tag="wv")
                        zt = kvp.tile([128, 8, KV], FP16, tag="zt")
                        for c in range(8):
                            nc.scalar.dma_start(wk_sb[:, c], wkv[c])
                        nc.sync.dma_start(zt[:], zTv)
                        for c in range(8):
                            nc.scalar.dma_start(wv_sb[:, c], wvv[c])
                        for c in range(8):
                            nc.sync.dma_start(wq_sb[:, c], wqv[c])
                        qt0_pre = attpre.tile([128, 8, SUB], FP16, tag="qt0")
                        nc.sync.dma_start(qt0_pre[:], qTv[:, :, 0:SUB])

                        # LN stats + normalize z per 512-token half
                        for hf in range(2):
                            sl = slice(hf * 512, hf * 512 + 512)
                            ur_z = stat_rows(kvp, psStat, psSC, kvp,
                                             zt[:, :, sl], 8, 512, "z",
                                             rows="ur")
                            t1 = kvp.tile([128, 8, 512], FP16, tag="zn1")
                            nc.vector.tensor_tensor(
                                t1[:], zt[:, :, sl],
                                ur_z[:, 0:1, :].to_broadcast((128, 8, 512)),
                                op=ALU.subtract)
                            nc.vector.tensor_tensor(
                                zt[:, :, sl], t1[:],
                                ur_z[:, 1:2, :].to_broadcast((128, 8, 512)),
                                op=ALU.mult)
                        for h in range(HPC):
                            for hf in range(2):
                                sl = slice(hf * 512, hf * 512 + 512)
                                kps = acc_ps(2 * h + hf)
                                for c in range(8):
                                    nc.tensor.matmul(
                                        kps[:], wk_sb[:, c, 128 * h:128 * h + 128],
                                        zt[:, c, sl], start=(c == 0), stop=(c == 7))
                                nc.vector.tensor_copy(K_sb[:, h, sl], kps[:])
                        if qbias:
                            for h in range(HPC):
                                for c in range(8):
                                    cps = psDen.tile([128, 1], F32, tag="den")
                                    nc.tensor.matmul(
                                        cps[:], K_sb[:, h, 128 * c:128 * c + 128],
                                        bq16[:, h:h + 1], start=True, stop=True)
                                    nc.vector.tensor_copy(c1_sb[:, h, c:c + 1],
                                                          cps[:])
                        for kc in range(8):
                            vps = acc_ps(kc)
                            for c in range(8):
                                nc.tensor.matmul(
                                    vps[:], zt[:, c, 128 * kc:128 * kc + 128],
                                    wv_sb[:, c], start=(c == 0), stop=(c == 7))
                            nc.vector.tensor_tensor(
                                V_sb[:, kc], vps[:], bvb_t[:], op=ALU.add)
                        # prefetch Wo during attention (scalar queue)
                        for c in range(8):
                            nc.scalar.dma_start(wo_sb[:, c], wov[c])

                    # ========== attention (q-stats fused, sw-pipelined) ====
                    with nc.named_scope("att"), \
                         tc.tile_pool(name="attp", bufs=3) as attp, \
                         tc.tile_pool(name="stp", bufs=1) as stp, \
                         tc.tile_pool(name="stp2", bufs=2) as stp2, \
                         tc.tile_pool(name="qhp", bufs=3) as qhp, \
                         tc.tile_pool(name="qr2", bufs=2) as qr2, \
                         tc.tile_pool(name="pcp", bufs=3) as pcp:
                        qts, sqs, rms = {}, {}, {}

                        def att_load(i):
                            qt = attp.tile([128, 8, SUB], FP16, tag="qt")
                            nc.sync.dma_start(qt[:],
                                              qTv[:, :, SUB * i:SUB * (i + 1)])
                            qts[i] = qt

                        def att_square(i):
                            sqs[i] = stat_square(stp2, qts[i][:], 8, SUB, "q")

                        def att_stat_mid(i):
                            st = stat_mms(psStat, qts[i][:], sqs.pop(i)[:],
                                          8, SUB)
                            return stat_rowmath(stp, st, D, SUB, "q")

                        def att_head(i, h, qt, rm_i):
                            s, isub = divmod(i, 8)
                            P = psP.tile([128, SUB], F32, tag="P")
                            for c in range(8):
                                nc.tensor.matmul(
                                    P[:], wq_sb[:, c, 128 * h:128 * h + 128],
                                    qt[:, c], start=(c == 0), stop=(c == 7))
                            qa = qr2.tile([128, SUB], FP16, tag="qa")
                            nc.vector.tensor_tensor(qa[:], P[:], rm_i[:, 0],
                                                    op=ALU.mult)
                            Qh = qhp.tile([128, SUB], FP16, tag="Qh")
                            nc.vector.scalar_tensor_tensor(
                                Qh[:], rm_i[:, 1], nwq_t[:, h:h + 1],
                                qa[:], op0=ALU.mult, op1=ALU.add)
                            ops = psOps.tile([128, SUB], F32, tag="ops")
                            pcs = []
                            pts = []
                            for g in range(4):
                                scp = psSC.tile([128, 2, SUB], F32, tag="sc")
                                for j in range(2):
                                    c = 2 * g + j
                                    nc.tensor.matmul(
                                        scp[:, j],
                                        K_sb[:, h, 128 * c:128 * c + 128],
                                        Qh[:], start=True, stop=True)
                                pc = pcp.tile([128, 2, SUB], FP16, tag="pc")
                                if qbias:
                                    for j in range(2):
                                        c = 2 * g + j
                                        nc.scalar.activation(
                                            pc[:, j], scp[:, j], AF.Exp,
                                            bias=c1_sb[:, h, c:c + 1])
                                else:
                                    nc.scalar.activation(pc[:], scp[:],
                                                         AF.Exp)
                                pcs.append(pc)
                                for j in range(2):
                                    c = 2 * g + j
                                    nc.tensor.matmul(
                                        ops[:],
                                        V_sb[:, c, 128 * h:128 * h + 128],
                                        pc[:, j], start=(c == 0),
                                        stop=(c == 7))
                                if g % 2 == 1:
                                    pt = pcp.tile([128, 2, SUB], FP16,
                                                  tag="pt")
                                    nc.vector.tensor_tensor(
                                        pt[:], pcs[-2][:], pcs[-1][:],
                                        op=ALU.add)
                                    pts.append(pt)
                            r8 = pcp.tile([128, 2, SUB], FP16, tag="pt")
                            nc.vector.tensor_tensor(r8[:], pts[0][:],
                                                    pts[1][:], op=ALU.add)
                            p8 = qr2.tile([128, SUB], FP16, tag="p8")
                            nc.vector.tensor_tensor(p8[:], r8[:, 0],
                                                    r8[:, 1], op=ALU.add)
                            den = psDen.tile([128, SUB], F32, tag="den")
                            nc.tensor.matmul(den[:], ones128[:], p8[:],
                                             start=True, stop=True)
                            rec = qr2.tile([128, SUB], F32, tag="rec")
                            nc.vector.reciprocal(rec[:], den[:])
                            nc.vector.tensor_tensor(
                                O_str[(s, h)][:, SUB * isub:SUB * (isub + 1)],
                                ops[:], rec[:], op=ALU.mult)

                        # prologue: qt 0 pre-loaded during kv
                        qts[0] = qt0_pre
                        att_load(1)
                        att_square(0)
                        att_square(1)
                        ra0, rb0 = att_stat_mid(0)
                        rms[0] = stat_repl(psSC, qr2, ra0, rb0, SUB, "q")

                        for i in range(NSUB):
                            qt = qts[i]
                            rm_i = rms.pop(i)
                            if i + 2 < NSUB:
                                att_load(i + 2)
                            rows_n = None
                            for h in range(HPC):
                                if h == 1 and i + 2 < NSUB:
                                    att_square(i + 2)
                                if h == 3 and i + 1 < NSUB:
                                    rows_n = att_stat_mid(i + 1)
                                att_head(i, h, qt, rm_i)
                            qts.pop(i)
                            if i + 1 < NSUB:
                                rms[i + 1] = stat_repl(psSC, qr2, rows_n[0],
                                                       rows_n[1], SUB, "q")

                    # ============ Wo + attn LN (sw-pipelined) ============
                    with nc.named_scope("wo"), \
                         tc.tile_pool(name="wop", bufs=3) as wop, \
                         tc.tile_pool(name="wox", bufs=2) as wox, \
                         tc.tile_pool(name="wst", bufs=1) as wst, \
                         tc.tile_pool(name="wst2", bufs=2) as wst2:
                        # preload the Gelu ACT table while ACT is idle here
                        # (saves the ~2.7us table swap at the p2 seam)
                        gpre = wst.tile([1, 8], F32, tag="gpre")
                        nc.scalar.activation(gpre[:], bo_t[0:1, :], AF.Gelu)
                        AOs, wsqs = {}, {}

                        def wo_front(t):
                            s, h = divmod(t, HPC)
                            AO = wop.tile([128, 8, SUB], FP16, tag="AO")
                            for oc in range(8):
                                aps = acc_ps(oc)
                                for u in range(8):
                                    nc.tensor.matmul(
                                        aps[:],
                                        wo_sb[:, u, 128 * oc:128 * oc + 128],
                                        O_str[(s, h)][:, SUB * u:SUB * (u + 1)],
                                        start=(u == 0), stop=(u == 7))
                                nc.scalar.activation(
                                    AO[:, oc], aps[:], AF.Identity,
                                    bias=bo_t[:, oc:oc + 1])
                            nc.gpsimd.dma_start(ao_stg[t], AO[:])
                            AOs[t] = AO
                            wsqs[t] = stat_square(wst2, AO[:], 8, SUB, "a")

                        def wo_back(t):
                            AO = AOs.pop(t)
                            st = stat_mms(psStat, AO[:], wsqs.pop(t)[:],
                                          8, SUB)
                            mu_a, rr_a = stat_rowmath(wst, st, D, SUB, "a",
                                                      rows="ur")
                            ur_a = stat_repl(psSC, wox, mu_a, rr_a, SUB, "a")
                            t1 = wox.tile([128, 8, SUB], FP16, tag="xn")
                            xn = wox.tile([128, 8, SUB], FP16, tag="xn")
                            nc.vector.tensor_tensor(
                                t1[:], AO[:],
                                ur_a[:, 0:1, :].to_broadcast((128, 8, SUB)),
                                op=ALU.subtract)
                            nc.vector.tensor_tensor(
                                xn[:], t1[:],
                                ur_a[:, 1:2, :].to_broadcast((128, 8, SUB)),
                                op=ALU.mult)
                            nc.gpsimd.dma_start(xn_stg[t], xn[:])

                        for t in range(8):
                            wo_front(t)
                            if t >= 2:
                                wo_back(t - 2)
                        wo_back(6)
                        wo_back(7)

              # ================= MLP + final projection =================
              with nc.named_scope("p2"), \
                   tc.tile_pool(name="w1p", bufs=1) as w1p, \
                   tc.tile_pool(name="w2p", bufs=1) as w2p, \
                   tc.tile_pool(name="p2h", bufs=1) as p2h, \
                   tc.tile_pool(name="p2b", bufs=2) as p2b, \
                   tc.tile_pool(name="p2ao", bufs=1) as p2ao, \
                   tc.tile_pool(name="p2ps", bufs=4, space="PSUM") as p2ps, \
                   tc.tile_pool(name="p2psx", bufs=1, space="PSUM") as p2psx:
                # chunked by G-groups of 4 so t=0's matmuls start early
                w1_sb = w1p.tile([128, 8, HID], FP16, tag="w1")
                for gg in range(8):
                    nc.scalar.dma_start(w1_sb[:, :, 512 * gg:512 * (gg + 1)],
                                        w1v[:, :, 512 * gg:512 * (gg + 1)])
                w2_sb = w2p.tile([128, 32, D], FP16, tag="w2")
                for c in range(32):
                    nc.scalar.dma_start(w2_sb[:, c], w2v[c])
                wf_sb = w2p.tile([128, 8, OUT_C], FP16, tag="wf")
                for t in range(8):
                    s2, h2 = divmod(t, HPC)
                    rowoff = 1024 * h2 + 512 * s2
                    xn_t = p2b.tile([128, 8, SUB], FP16, tag="xnin")
                    nc.sync.dma_start(xn_t[:], xn_stg[t])
                    ao_t = p2ao.tile([128, 8, SUB], FP16, tag="aot")
                    nc.sync.dma_start(ao_t[:], ao_stg[t])
                    if t == 0:
                        for c in range(8):
                            nc.sync.dma_start(wf_sb[:, c], wfv[c])
                    h_sb = p2h.tile([128, 32, SUB], FP16, tag="h")
                    for G in range(32):
                        hps = p2ps.tile([128, SUB], F32, tag="hps")
                        for c in range(8):
                            nc.tensor.matmul(
                                hps[:],
                                w1_sb[:, c, 128 * G:128 * G + 128],
                                xn_t[:, c], start=(c == 0), stop=(c == 7))
                        nc.scalar.activation(h_sb[:, G], hps[:], AF.Gelu,
                                             bias=b1_t[:, G:G + 1])
                    X = p2h.tile([128, 8, SUB], FP16, tag="X")
                    for half in range(2):
                        xps = p2psx.tile([128, 4, SUB], F32, tag="xps")
                        for G in range(32):
                            for oc4 in range(4):
                                oc = 4 * half + oc4
                                nc.tensor.matmul(
                                    xps[:, oc4],
                                    w2_sb[:, G, 128 * oc:128 * oc + 128],
                                    h_sb[:, G], start=(G == 0), stop=(G == 31))
                        for oc4 in range(4):
                            oc = 4 * half + oc4
                            nc.vector.scalar_tensor_tensor(
                                X[:, oc], xps[:, oc4], b2_t[:, oc:oc + 1],
                                ao_t[:, oc],
                                op0=ALU.add, op1=ALU.add)
                    for of in range(4):
                        ofps = p2ps.tile([128, SUB], F32, tag="hps")
                        for c in range(8):
                            nc.tensor.matmul(
                                ofps[:], wf_sb[:, c, 128 * of:128 * of + 128],
                                X[:, c], start=(c == 0), stop=(c == 7))
                        outt = p2b.tile([128, SUB], FP16, tag="outt")
                        nc.vector.tensor_scalar_add(outt[:], ofps[:],
                                                    bf_t[:, of:of + 1])
                        nc.sync.dma_start(
                            outT[128 * of:128 * (of + 1),
                                 rowoff:rowoff + SUB], outt[:])
    nc.compile()
    return nc


def _prep_host(inputs):
    """Fold LN gains + attention scale into weights; build per-core maps."""
    f64 = np.float64
    gq, bq_ln = inputs["ln_q_g"].astype(f64), inputs["ln_q_b"].astype(f64)
    gkv = inputs["ln_kv_g"].astype(f64)
    bkv_ln = inputs["ln_kv_b"].astype(f64)
    ga, ba_ln = inputs["ln_a_g"].astype(f64), inputs["ln_a_b"].astype(f64)
    Wq, Wk, Wv = (np.asarray(inputs[k], f64) for k in ("Wq", "Wk", "Wv"))
    Wo, W1, W2, Wf = (np.asarray(inputs[k], f64) for k in ("Wo", "W1", "W2", "Wf"))
    bq_, bv_ = (np.asarray(inputs[k], f64) for k in ("bq", "bv"))
    bo_, b1_, b2_, bf_ = (np.asarray(inputs[k], f64)
                          for k in ("bo", "b1", "b2", "bf"))

    sc = 1.0 / np.sqrt(DH)
    Wq_e = (gq[:, None] * Wq) * sc
    bq_e = (bq_ln @ Wq + bq_) * sc
    Wk_e = gkv[:, None] * Wk
    Wv_e = gkv[:, None] * Wv
    bv_e = bkv_ln @ Wv + bv_
    W1_e = ga[:, None] * W1
    b1_e = ba_ln @ W1 + b1_

    qbias = bool(np.abs(bq_e).max() > 1e-12)

    perm = _query_perm()
    f32 = np.float32
    query = np.asarray(inputs["query"], f32)
    z = np.asarray(inputs["z"], f32)
    maps = []
    shared = {
        "wo": np.ascontiguousarray(Wo.astype(np.float16)),
        "w1": np.ascontiguousarray(W1_e.astype(np.float16)),
        "w2": np.ascontiguousarray(W2.astype(np.float16)),
        "wf": np.ascontiguousarray(Wf.astype(np.float16)),
        "bo": np.ascontiguousarray(bo_.reshape(8, 128).T.astype(f32)),
        "b1": np.ascontiguousarray(b1_e.reshape(32, 128).T.astype(f32)),
        "b2": np.ascontiguousarray(b2_.reshape(8, 128).T.astype(f32)),
        "bfp": np.ascontiguousarray(bf_.reshape(4, 128).T.astype(f32)),
    }
    for core in range(N_CORES):
        b, g = divmod(core, 2)
        hs = slice(512 * g, 512 * (g + 1))
        m = dict(shared)
        nwq_v = -(Wq_e[:, hs].sum(axis=0))
        m.update({
            "qT": np.ascontiguousarray(query[b][perm].T.astype(np.float16)),
            "zT": np.ascontiguousarray(z[b].T.astype(np.float16)),
            "wq": np.ascontiguousarray(Wq_e[:, hs].astype(np.float16)),
            "wk": np.ascontiguousarray(Wk_e[:, hs].astype(np.float16)),
            "wv": np.ascontiguousarray(Wv_e[:, hs].astype(np.float16)),
            "nwq": np.ascontiguousarray(nwq_v.reshape(HPC, 128).T.astype(f32)),
            "bvb": np.broadcast_to(bv_e[hs].astype(f32), (128, 512)).copy(),
        })
        if qbias:
            m["bqh"] = np.ascontiguousarray(
                bq_e[hs].reshape(HPC, 128).T.astype(f32))
        maps.append(m)
    return maps, qbias


def kernel(**inputs):
    assert bool(np.all(inputs["query_mask"])), \
        "kernel specialization assumes all-ones query_mask"
    maps, qbias = _prep_host(inputs)
    key = ("nc", qbias)
    if key not in _CACHE:
        _CACHE[key] = build(qbias=qbias)
    nc = _CACHE[key]
    res = bass_utils.run_bass_kernel_spmd(nc, maps, core_ids=list(range(N_CORES)))
    out = np.empty((B, Q, OUT_C), dtype=np.float32)
    for core in range(N_CORES):
        b, g = divmod(core, 2)
        out[b, ROWS * g:ROWS * (g + 1), :] = res.results[core]["outT"].T
    return out
